# revision 13
# baseline (speedup 1.0000x reference)
"""Trainium2 Bass kernel for nn_MILPAttention (dense multi-head attention with
per-key additive bias), tensor-parallel over heads across 8 NeuronCores.

Self-contained: hardcodes shapes N=4096, D=1024, H=16, GAMMA=1.0.

Math (reference):
    q = x @ Wq.T + bq ; k = x @ Wk.T + bk ; v = x @ Wv.T + bv     (per head, dh=64)
    logits = (q @ k.T) / 8 - h[key]
    attn = softmax(logits, keys)
    out = (attn @ v) @ Wo.T + bo + x

Per-core strategy (core i owns heads 2i, 2i+1 = columns 128i:128(i+1)):
  - Projections computed transposed: qT,kT [128, 4096] = W.T.T @ x.T with the
    1/8 scale folded into Wq/bq on the host. v is transposed to natural
    [keys, 64] layout via PE transposes and pre-scaled by 0.25*exp(-h) (folds
    the per-key softmax bias into V; the 0.25 keeps fp8 operands in range);
    the same 0.25*exp(-h) is appended as a 65th lhs column so each P@V matmul
    also yields the softmax denominator.
  - S^T[key, q] = kT.T @ qT per head (K=64 contraction); the two heads run
    concurrently in disjoint PE row groups.  A static offset C=5 is folded
    into exp so weights fit fp8 range; it cancels in the normalization.
  - exp is split across engines by key-chunk class (kc % 8), alternating so
    ScalarE and VectorE run concurrently:
      0,2,4,6: ScalarE exp -> e4m3; P@V runs as fp8 DoubleRow matmuls with
               chunk pairs (0,2) and (4,6) interleaved in the contraction
               (K=256 per matmul).
      1,3,5:   VectorE computes exp via the Schraudolph bit trick (int16
               bitcast to bf16), classic bf16 P@V.
      7:       ScalarE exp -> bf16, classic bf16 P@V.
  - P@V matmul emission lags the S/exp stream by a few chunks so the
    in-order PE queue never stalls waiting for an exp to finish.
  - Normalization snaps PSUM to SBUF, then reciprocal + partition broadcast
    + multiply into per-head [64, 512] tiles DMA'd to the all-to-all buffer
    per q-block; AllToAll switches from head-sharding to sequence-sharding
    and the output projection + bias + residual run on this core's 512 rows.
"""
from collections import deque
import numpy as np

import concourse.bass as bass
import concourse.mybir as mybir
import concourse.tile as tile
from concourse import bacc
from concourse.bass_utils import run_bass_kernel_spmd
from concourse.masks import make_identity

N, D, H = 4096, 1024, 16
NCORE = 8
CB = D // NCORE          # 128 columns (2 heads) per core
NR = N // NCORE          # 512 output rows per core
DH = D // H              # 64
KCH = N // 128           # 32 key chunks
NB = N // 512            # 8 n-blocks
FP = mybir.dt.float32
BF = mybir.dt.bfloat16
E4 = mybir.dt.float8e4
I16 = mybir.dt.int16
AF = mybir.ActivationFunctionType
ALU = mybir.AluOpType
DR = mybir.MatmulPerfMode.DoubleRow

C_OFF = 5.0                      # static exp offset (cancels in softmax)
SCH_A = np.float32(184.6650)     # 128/ln(2)
SCH_B = np.float32(16256.0 - 4.0 - 184.6650 * C_OFF)
PV_LAG = 4                       # P@V emissions held back behind S/exp


def kc_cls(kc):
    m = kc % 8
    if m in (0, 2, 4, 6):
        return "A"               # ScalarE exp -> e4m3, fp8 DR P@V
    return "B" if m == 7 else "C"  # B: ScalarE bf16; C: VectorE schraudolph


def kc_pair_slot(kc):            # A chunks: pairs (0,2) and (4,6) per group
    m = kc % 8
    return (kc // 8) * 2 + (1 if m >= 4 else 0), (m % 4) // 2


def kc_b16_slot(kc):             # B/C chunks m in {1,3,5,7} -> 0..15
    m = kc % 8
    return (kc // 8) * 4 + m // 2


def _body(nc, tc, reps, xt, xr, wqt, wkt, wvt, wot, bqv, bkv, bvv, bov, hv, out,
          dbg=None, use_collective=True):
    cst = tc.alloc_tile_pool(name="cst", bufs=1)
    per = tc.alloc_tile_pool(name="per", bufs=1)
    dram = tc.alloc_tile_pool(name="dram", bufs=1, space="DRAM")

    ident = cst.tile([128, 128], BF)
    make_identity(nc, ident[:])

    # persistent sbuf
    wq_b = per.tile([128, D], BF)        # [d-in-chunk, dc*128 + c]
    wk_b = per.tile([128, D], BF)
    wv_b = per.tile([128, D], BF)
    wo_b = per.tile([128, 8 * D], BF)    # [c-in-chunk, cc*1024 + o]
    qb_t = per.tile([128, N], BF)        # qT: rows = 2 heads x 64 dims
    kb_t = per.tile([128, N], BF)
    # fp8 DR lhs: per pair slot 2x144 cols: [v_h0(64) w v_h1(64) w pad(14)]
    vw8 = per.tile([128, 8 * 2 * 144], E4)
    # bf16 lhs: per chunk slot 130 cols: [v_h0(64) w v_h1(64) w]
    vwb = per.tile([128, 16 * 130], BF)
    bq_s = per.tile([128, 1], FP)
    bk_s = per.tile([128, 1], FP)
    bv_s = per.tile([128, 1], FP)
    w_s2 = per.tile([128, KCH], FP)      # 0.25*exp(-h), [key-in-chunk, chunk]
    xb_s = [per.tile([128, D], BF, name=f"xb{j}") for j in range(4)]  # x rows + bo
    xts = [per.tile([128, N], BF, name=f"xts{dc}") for dc in range(8)]  # full x^T

    cc_in = dram.tile([NCORE * 128, NR], BF)
    cc_out = dram.tile([NCORE * 128, NR], BF)

    vw8v = vw8[:].rearrange("p (s j m) -> p s j m", j=2, m=144)

    for rep in range(reps):
        sfx = f"_{rep}"
        # ---------------- phase 0: constants --------------------------------
        # wk + the first x columns first: the k-projection chain is the
        # first PE consumer
        nc.sync.dma_start(wk_b[:].rearrange("p (dc c) -> p dc c", c=CB),
                          wkt.rearrange("(dc p) c -> p dc c", p=128))
        nc.scalar.dma_start(wv_b[:].rearrange("p (dc c) -> p dc c", c=CB),
                            wvt.rearrange("(dc p) c -> p dc c", p=128))
        nc.scalar.dma_start(wq_b[:].rearrange("p (dc c) -> p dc c", c=CB),
                            wqt.rearrange("(dc p) c -> p dc c", p=128))
        hst = per.tile([128, KCH], FP, name="hst")
        nc.scalar.dma_start(hst[:], hv)
        negc = per.tile([128, 1], FP, name="negc")
        nc.vector.memset(negc[:], -C_OFF)
        ln14 = per.tile([128, 1], FP, name="ln14")
        nc.vector.memset(ln14[:], float(np.log(0.25)))
        # 0.25*exp(-h) = exp(-h + ln(0.25))
        nc.scalar.activation(w_s2[:], hst[:], AF.Exp, scale=-1.0,
                             bias=ln14[:, 0:1])
        nc.sync.dma_start(bq_s[:], bqv.unsqueeze(1))
        nc.scalar.dma_start(bk_s[:], bkv.unsqueeze(1))
        nc.scalar.dma_start(bv_s[:], bvv.unsqueeze(1))

        # ------- phases 1+2 fully woven ------------------------------------
        with tc.tile_pool(name="p1s" + sfx, bufs=2) as p1s, \
             tc.tile_pool(name="p1p", bufs=1, space="PSUM") as p1p, \
             tc.tile_pool(name="p8s" + sfx, bufs=3) as p8s, \
             tc.tile_pool(name="pbs" + sfx, bufs=5) as pbs, \
             tc.tile_pool(name="paos" + sfx, bufs=2) as paos, \
             tc.tile_pool(name="pfp" + sfx, bufs=2) as pfp, \
             tc.tile_pool(name="p2ps", bufs=2, space="PSUM") as p2ps, \
             tc.tile_pool(name="p2po", bufs=1, space="PSUM") as p2po:

            def block_load(nb):
                ncol = slice(nb * 512, (nb + 1) * 512)
                for dc in range(8):
                    nc.sync.dma_start(
                        xts[dc][:, ncol],
                        xt[dc * 128:(dc + 1) * 128, nb * 512:(nb + 1) * 512])

            def kv_compute_k(nb):
                ncol = slice(nb * 512, (nb + 1) * 512)
                psk = p1p.tile([128, 512], FP, name="proj")
                for dc in range(8):
                    nc.tensor.matmul(psk[:], wk_b[:, dc * CB:(dc + 1) * CB],
                                     xts[dc][:, ncol],
                                     start=(dc == 0), stop=(dc == 7))
                nc.scalar.add(kb_t[:, nb * 512:(nb + 1) * 512], psk[:],
                              bk_s[:, 0:1])

            def kv_compute_v(nb):
                ncol = slice(nb * 512, (nb + 1) * 512)
                psv = p1p.tile([128, 512], FP, name="pvt")
                for dc in range(8):
                    nc.tensor.matmul(psv[:], wv_b[:, dc * CB:(dc + 1) * CB],
                                     xts[dc][:, ncol],
                                     start=(dc == 0), stop=(dc == 7))
                vtb = p1s.tile([128, 512], BF, name="vtb")
                nc.scalar.add(vtb[:], psv[:], bv_s[:, 0:1])
                for ns in range(4):
                    kc = nb * 4 + ns
                    pvt = p1p.tile([128, 128], BF, name="pvt")
                    nc.tensor.transpose(pvt[:], vtb[:, ns * 128:(ns + 1) * 128], ident[:])
                    wcol = w_s2[:, kc:kc + 1]
                    if kc_cls(kc) == "A":
                        p8, j = kc_pair_slot(kc)
                        base = (p8 * 2 + j) * 144
                        dst = vw8[:, base:base + 130]
                    else:
                        base = kc_b16_slot(kc) * 130
                        dst = vwb[:, base:base + 130]
                    nc.vector.tensor_scalar_mul(dst[:, 0:64], pvt[:, 0:64], wcol)
                    nc.vector.tensor_copy(dst[:, 64:65], wcol)
                    nc.vector.tensor_scalar_mul(dst[:, 65:129], pvt[:, 64:128], wcol)
                    nc.vector.tensor_copy(dst[:, 129:130], wcol)

            def qp_compute(nb):
                ncol = slice(nb * 512, (nb + 1) * 512)
                psq = p1p.tile([128, 512], FP, name="proj")
                for dc in range(8):
                    nc.tensor.matmul(psq[:], wq_b[:, dc * CB:(dc + 1) * CB],
                                     xts[dc][:, ncol],
                                     start=(dc == 0), stop=(dc == 7))
                nc.vector.tensor_scalar_add(qb_t[:, nb * 512:(nb + 1) * 512],
                                            psq[:], bq_s[:, 0:1])

            block_load(0)
            block_load(1)
            kv_compute_k(0)
            kv_compute_v(0)
            qp_compute(0)

            # prefetch phase-3 constants (issued on the mostly-idle gpsimd
            # queue); transfers overlap the attention phase
            nc.gpsimd.dma_start(wo_b[:].rearrange("p (cc o) -> p cc o", o=D),
                                wot.rearrange("(cc p) o -> p cc o", p=128))
            bost = pfp.tile([128, D], FP, name="bost")
            nc.gpsimd.dma_start(bost[:], bov.unsqueeze(0).broadcast_to([128, D]))
            for j in range(4):
                xrt = pfp.tile([128, D], FP, name="xrt")
                nc.gpsimd.dma_start(xrt[:], xr[j * 128:(j + 1) * 128, :])
                nc.vector.tensor_add(xb_s[j][:], xrt[:], bost[:])

            kv_next, qp_next = 1, 1

            pending = deque()
            norm_todo = []

            def emit_pv():
                pending.popleft()()

            def mk_dr(pso_, idx_, p8, pb8v_):
                def go():
                    first = idx_[0] == 0
                    idx_[0] += 2
                    last = idx_[0] == 24
                    for h in range(2):
                        nc.tensor.matmul(
                            pso_[h][:], vw8v[:, p8, :, h * 65:(h + 1) * 65],
                            pb8v_[:, :, h * 512:(h + 1) * 512],
                            start=first, stop=last, perf_mode=DR)
                return go

            def mk_b16(pso_, idx_, kc, pbb_):
                b0 = kc_b16_slot(kc) * 130
                def go():
                    first = idx_[0] == 0
                    idx_[0] += 1
                    last = idx_[0] == 24
                    for h in range(2):
                        nc.tensor.matmul(
                            pso_[h][:], vwb[:, b0 + h * 65: b0 + (h + 1) * 65],
                            pbb_[:, h * 512:(h + 1) * 512],
                            start=first, stop=last)
                return go

            def normalize(pso_, q5_):
                qc = slice(q5_ * 512, (q5_ + 1) * 512)
                for h in range(2):
                    snap = paos.tile([65, 512], FP, name=f"sn{h}")
                    nc.vector.tensor_copy(snap[:], pso_[h][:])
                    rc = paos.tile([1, 512], FP, name=f"rc{h}")
                    nc.vector.reciprocal(rc[:], snap[64:65, :])
                    bch = paos.tile([64, 512], FP, name=f"bc{h}")
                    nc.gpsimd.partition_broadcast(bch[:], rc[:])
                    aoh = paos.tile([64, 512], BF, name=f"ao{h}")
                    nc.vector.tensor_tensor(aoh[:], snap[0:64, :], bch[:], ALU.mult)
                    nc.sync.dma_start(
                        cc_in[q5_ * 128 + h * 64: q5_ * 128 + (h + 1) * 64, :],
                        aoh[:])
                    if dbg is not None:
                        nc.sync.dma_start(
                            dbg["d_ao"][h * 64:(h + 1) * 64, qc], aoh[:])

            for q5 in range(N // 512):
                qcol = slice(q5 * 512, (q5 + 1) * 512)
                pso = [p2po.tile([65, 512], FP, name=f"pso{h}") for h in range(2)]
                pb8 = None
                pv_idx = [0]

                for kc in range(KCH):
                    krng = slice(kc * 128, (kc + 1) * 128)
                    cls = kc_cls(kc)
                    pss = p2ps.tile([128, 1024], FP, name="pss")
                    for h in range(2):
                        hr = slice(h * 64, (h + 1) * 64)
                        nc.tensor.matmul(pss[:, h * 512:(h + 1) * 512],
                                         kb_t[hr, krng], qb_t[hr, qcol],
                                         start=True, stop=True)
                    if cls == "A":
                        p8, j = kc_pair_slot(kc)
                        if j == 0:
                            pb8 = p8s.tile([128, 2048], E4, name="pb8")
                        pb8v = pb8[:].rearrange("p (j n) -> p j n", j=2)
                        nc.scalar.activation(pb8v[:, j, :], pss[:], AF.Exp,
                                             bias=negc[:, 0:1])
                        if j == 1:
                            pending.append(mk_dr(pso, pv_idx, p8, pb8v))
                    else:
                        pbb = pbs.tile([128, 1024], BF, name="pbb")
                        if cls == "B":
                            nc.scalar.activation(pbb[:], pss[:], AF.Exp,
                                                 bias=negc[:, 0:1])
                        else:
                            nc.vector.tensor_scalar(
                                pbb[:].bitcast(I16), pss[:],
                                float(SCH_A), float(SCH_B), ALU.mult, ALU.add)
                        pending.append(mk_b16(pso, pv_idx, kc, pbb))
                    while len(pending) > PV_LAG:
                        emit_pv()
                    # previous q-block's normalize: its last P@V flushed by
                    # kc==5, before this block's first P@V is emitted
                    if kc == 5 and norm_todo:
                        normalize(*norm_todo.pop())
                    # weave projection blocks under the exp stream
                    if q5 == 0:
                        if kc % 4 == 0 and kc // 4 + 2 < NB:
                            block_load(kc // 4 + 2)
                        if kc % 4 == 1 and kv_next < NB:
                            kv_compute_k(kv_next)
                        if kc % 4 == 2 and kv_next < NB:
                            kv_compute_v(kv_next)
                            kv_next += 1
                        if kc == 3 and qp_next < NB:
                            qp_compute(qp_next)
                            qp_next += 1
                    if q5 in (1, 2):
                        if kc % 12 == 4 and qp_next < NB:
                            qp_compute(qp_next)
                            qp_next += 1
                if q5 < N // 512 - 1:
                    norm_todo.append((pso, q5))
                else:
                    while pending:
                        emit_pv()
                    normalize(pso, q5)

        # ---------------- phase 3: A2A + out projection ----------------
        with tc.tile_pool(name="p3s" + sfx, bufs=1) as p3s, \
             tc.tile_pool(name="p3f" + sfx, bufs=3) as p3f, \
             tc.tile_pool(name="p3p", bufs=1, space="PSUM") as p3p:
            if use_collective:
                nc.gpsimd.collective_compute(
                    "AllToAll", mybir.AluOpType.bypass,
                    replica_groups=[list(range(NCORE))],
                    ins=[cc_in[:].opt()], outs=[cc_out[:].opt()])
            else:  # single-core timing-sim stand-in
                nc.sync.dma_start(cc_out[:], cc_in[:])
            psf = [p3p.tile([128, 512], FP, name=f"psf{t}") for t in range(8)]
            aocs = []
            for cc in range(8):
                aoc = p3s.tile([128, NR], BF, name=f"aoc{cc}")
                (nc.sync if cc % 2 == 0 else nc.scalar).dma_start(
                    aoc[:], cc_out[cc * 128:(cc + 1) * 128, :])
                if dbg is not None:
                    nc.sync.dma_start(dbg["d_aoc"][cc * 128:(cc + 1) * 128, :], aoc[:])
                aocs.append(aoc)
            for ns in range(4):
                for ob in range(2):
                    t = ns * 2 + ob
                    for cc in range(8):
                        nc.tensor.matmul(
                            psf[t][:],
                            aocs[cc][:, ns * 128:(ns + 1) * 128],
                            wo_b[:, cc * D + ob * 512: cc * D + (ob + 1) * 512],
                            start=(cc == 0), stop=(cc == 7))
                    fo = p3f.tile([128, 512], FP, name="fo")
                    nc.vector.tensor_add(fo[:], psf[t][:],
                                         xb_s[ns][:, ob * 512:(ob + 1) * 512])
                    nc.sync.dma_start(
                        out[ns * 128:(ns + 1) * 128, ob * 512:(ob + 1) * 512], fo[:])

    if dbg is not None:
        for nm, t in (("d_q", qb_t), ("d_k", kb_t), ("d_vwb", vwb)):
            nc.sync.dma_start(dbg[nm], t[:])
        nc.sync.dma_start(dbg["d_vw8"], vw8[:].bitcast(mybir.dt.uint8))
        nc.sync.dma_start(dbg["d_cc"], cc_out[:])

    dram.release()
    per.release()
    cst.release()


def build_nc(reps=1, debug=False, use_collective=True):
    nc = bacc.Bacc("TRN2", target_bir_lowering=False, debug=False, num_devices=NCORE)
    xt = nc.dram_tensor("xt", [D, N], BF, kind="ExternalInput").ap()
    xr = nc.dram_tensor("xr", [NR, D], FP, kind="ExternalInput").ap()
    wqt = nc.dram_tensor("wqt", [D, CB], BF, kind="ExternalInput").ap()
    wkt = nc.dram_tensor("wkt", [D, CB], BF, kind="ExternalInput").ap()
    wvt = nc.dram_tensor("wvt", [D, CB], BF, kind="ExternalInput").ap()
    wot = nc.dram_tensor("wot", [D, D], BF, kind="ExternalInput").ap()
    bqv = nc.dram_tensor("bqv", [CB], FP, kind="ExternalInput").ap()
    bkv = nc.dram_tensor("bkv", [CB], FP, kind="ExternalInput").ap()
    bvv = nc.dram_tensor("bvv", [CB], FP, kind="ExternalInput").ap()
    bov = nc.dram_tensor("bov", [D], FP, kind="ExternalInput").ap()
    # h pre-rearranged on host to [128, KCH]: hv[p, c] = h[c*128 + p]
    hv = nc.dram_tensor("hv", [128, KCH], FP, kind="ExternalInput").ap()
    out = nc.dram_tensor("out", [NR, D], FP, kind="ExternalOutput").ap()
    dbg = None
    if debug:
        dbg = {
            "d_q": nc.dram_tensor("d_q", [128, N], BF, kind="ExternalOutput").ap(),
            "d_k": nc.dram_tensor("d_k", [128, N], BF, kind="ExternalOutput").ap(),
            "d_vw8": nc.dram_tensor("d_vw8", [128, 8 * 2 * 144], mybir.dt.uint8,
                                    kind="ExternalOutput").ap(),
            "d_vwb": nc.dram_tensor("d_vwb", [128, 16 * 130], BF,
                                    kind="ExternalOutput").ap(),
            "d_ao": nc.dram_tensor("d_ao", [128, N], BF, kind="ExternalOutput").ap(),
            "d_cc": nc.dram_tensor("d_cc", [NCORE * 128, NR], BF,
                                   kind="ExternalOutput").ap(),
            "d_aoc": nc.dram_tensor("d_aoc", [NCORE * 128, NR], BF,
                                    kind="ExternalOutput").ap(),
        }
    with tile.TileContext(nc) as tc:
        _body(nc, tc, reps, xt, xr, wqt, wkt, wvt, wot,
              bqv, bkv, bvv, bov, hv, out, dbg=dbg, use_collective=use_collective)
    nc.compile()
    return nc


_NC_CACHE = {}


def get_nc(reps=1):
    if reps not in _NC_CACHE:
        _NC_CACHE[reps] = build_nc(reps)
    return _NC_CACHE[reps]


def make_in_maps(inputs):
    x = np.ascontiguousarray(np.asarray(inputs["x"], dtype=np.float32))
    h = np.ascontiguousarray(np.asarray(inputs["h"], dtype=np.float32))
    Wq = np.asarray(inputs["Wq"], dtype=np.float32)
    bq = np.asarray(inputs["bq"], dtype=np.float32)
    Wk = np.asarray(inputs["Wk"], dtype=np.float32)
    bk = np.asarray(inputs["bk"], dtype=np.float32)
    Wv = np.asarray(inputs["Wv"], dtype=np.float32)
    bv = np.asarray(inputs["bv"], dtype=np.float32)
    Wo = np.asarray(inputs["Wo"], dtype=np.float32)
    bo = np.ascontiguousarray(np.asarray(inputs["bo"], dtype=np.float32))
    import ml_dtypes
    bf16 = ml_dtypes.bfloat16
    xt = np.ascontiguousarray(x.T.astype(bf16))
    wot = np.ascontiguousarray(Wo.T.astype(bf16))
    scale = np.float32(0.125)  # 1/sqrt(dh), folded into q
    in_maps = []
    for i in range(NCORE):
        cs = slice(i * CB, (i + 1) * CB)
        in_maps.append({
            "xt": xt,
            "xr": np.ascontiguousarray(x[i * NR:(i + 1) * NR, :]),
            "wqt": np.ascontiguousarray((Wq[cs, :] * scale).T.astype(bf16)),
            "wkt": np.ascontiguousarray(Wk[cs, :].T.astype(bf16)),
            "wvt": np.ascontiguousarray(Wv[cs, :].T.astype(bf16)),
            "wot": wot,
            "bqv": np.ascontiguousarray(bq[cs] * scale),
            "bkv": np.ascontiguousarray(bk[cs]),
            "bvv": np.ascontiguousarray(bv[cs]),
            "bov": bo,
            "hv": np.ascontiguousarray(h.reshape(KCH, 128).T),
        })
    return in_maps


def kernel(**inputs):
    nc = get_nc(1)
    in_maps = make_in_maps(inputs)
    res = run_bass_kernel_spmd(nc, in_maps, core_ids=list(range(NCORE)))
    return np.concatenate([res.results[i]["out"] for i in range(NCORE)], axis=0)


# revision 19
# speedup vs baseline: 6.8807x; 6.8807x over previous
"""Trainium2 Bass kernel for nn_MILPAttention (dense multi-head attention with
per-key additive bias), tensor-parallel over heads across 8 NeuronCores.

Self-contained: hardcodes shapes N=4096, D=1024, H=16, GAMMA=1.0.

Math (reference):
    q = x @ Wq.T + bq ; k = x @ Wk.T + bk ; v = x @ Wv.T + bv     (per head, dh=64)
    logits = (q @ k.T) / 8 - h[key]
    attn = softmax(logits, keys)
    out = (attn @ v) @ Wo.T + bo + x

Per-core strategy (core i owns heads 2i, 2i+1 = columns 128i:128(i+1)):
  - Projections computed transposed: qT,kT [128, 4096] = W.T.T @ x.T with the
    1/8 scale folded into Wq/bq on the host. v is transposed to natural
    [keys, 64] layout via PE transposes and pre-scaled by 0.25*exp(-h) (folds
    the per-key softmax bias into V; the 0.25 keeps fp8 operands in range);
    the same 0.25*exp(-h) is appended as a 65th lhs column so each P@V matmul
    also yields the softmax denominator.
  - S^T[key, q] = kT.T @ qT per head (K=64 contraction); the two heads run
    concurrently in disjoint PE row groups.  A static offset C=5 is folded
    into exp so weights fit fp8 range; it cancels in the normalization.
  - exp is split across engines by key-chunk class (kc % 8), alternating so
    ScalarE and VectorE run concurrently:
      0,2,4,6: ScalarE exp -> e4m3; P@V runs as fp8 DoubleRow matmuls with
               chunk pairs (0,2) and (4,6) interleaved in the contraction
               (K=256 per matmul).
      1,3,5:   VectorE computes exp via the Schraudolph bit trick (int16
               bitcast to bf16), classic bf16 P@V.
      7:       ScalarE exp -> bf16, classic bf16 P@V.
  - P@V matmul emission lags the S/exp stream by a few chunks so the
    in-order PE queue never stalls waiting for an exp to finish.
  - Normalization snaps PSUM to SBUF, then reciprocal + partition broadcast
    + multiply into per-head [64, 512] tiles DMA'd to the all-to-all buffer
    per q-block; AllToAll switches from head-sharding to sequence-sharding
    and the output projection + bias + residual run on this core's 512 rows.
"""
from collections import deque
import numpy as np

import concourse.bass as bass
import concourse.mybir as mybir
import concourse.tile as tile
from concourse import bacc
from concourse.bass_utils import run_bass_kernel_spmd
from concourse.masks import make_identity

N, D, H = 4096, 1024, 16
NCORE = 8
CB = D // NCORE          # 128 columns (2 heads) per core
NR = N // NCORE          # 512 output rows per core
DH = D // H              # 64
KCH = N // 128           # 32 key chunks
NB = N // 512            # 8 n-blocks
FP = mybir.dt.float32
BF = mybir.dt.bfloat16
E4 = mybir.dt.float8e4
I16 = mybir.dt.int16
AF = mybir.ActivationFunctionType
ALU = mybir.AluOpType
DR = mybir.MatmulPerfMode.DoubleRow

A8 = 11.5416                     # 8/ln(2): e4m3 bits per nat
C_OFF = (56.0 + 0.5) / A8        # static exp offset chosen so the e4m3
                                 # Schraudolph needs no additive term; it
                                 # cancels in the softmax normalization
PV_LAG = 3                       # P@V pair emissions held back behind S/exp


def kc_cls(kc, sec_b=False):
    # chunk pairs (2i, 2i+1): even chunks -> ScalarE exp, odd -> VectorE u8
    # Schraudolph, plus one extra ScalarE chunk per 8 (per 16 in section B
    # where ScalarE also owns the snap copies); every chunk is e4m3 so all
    # P@V matmuls run as fp8 DoubleRow with K=256
    if kc % 2 == 0 or (kc % 16 == 7 if sec_b else kc % 8 == 7):
        return "A"
    return "C"


def kc_pair_slot(kc):
    return kc // 2, kc % 2


def _body(nc, tc, reps, xt, xr, wqt, wkt, wvt, wot, bqv, bkv, bvv, bov, hv, out,
          dbg=None, use_collective=True):
    cst = tc.alloc_tile_pool(name="cst", bufs=1)
    per = tc.alloc_tile_pool(name="per", bufs=1)
    dram = tc.alloc_tile_pool(name="dram", bufs=1, space="DRAM")

    ident = cst.tile([128, 128], BF)
    make_identity(nc, ident[:])

    # persistent sbuf
    wq_b = per.tile([128, D], BF)        # [d-in-chunk, dc*128 + c]
    wk_b = per.tile([128, D], BF)
    wv_b = per.tile([128, D], BF)
    wo_b = per.tile([128, 8 * D], E4)    # [c, (pair, j, o)] DR-interleaved
    qb_t = per.tile([128, N], BF)        # qT: rows = 2 heads x 64 dims
    kb_t = per.tile([128, N], BF)
    # fp8 DR lhs: per pair slot 2x144 cols: [v_h0(64) w v_h1(64) w pad(14)]
    vw8 = per.tile([128, 16 * 2 * 144], E4)
    bq_s = per.tile([128, 1], FP)
    bk_s = per.tile([128, 1], FP)
    bv_s = per.tile([128, 1], FP)
    w_s2 = per.tile([128, KCH], FP)      # 0.25*exp(-h), [key-in-chunk, chunk]
    xb_s = [per.tile([128, D], BF, name=f"xb{j}") for j in range(4)]  # x rows + bo
    xts = [per.tile([128, N], BF, name=f"xts{dc}") for dc in range(8)]  # full x^T

    cc_in = dram.tile([NCORE * 128, NR], E4)
    cc_out = dram.tile([NCORE * 128, NR], E4)

    vw8v = vw8[:].rearrange("p (s j m) -> p s j m", j=2, m=144)

    for rep in range(reps):
        sfx = f"_{rep}"
        # ---------------- phase 0: constants --------------------------------
        # wk + the first x columns first: the k-projection chain is the
        # first PE consumer; halves so the chain starts on the first half
        for half in range(2):
            hs, hd = slice(half * 512, (half + 1) * 512), slice(half * 4, (half + 1) * 4)
            nc.sync.dma_start(
                wk_b[:, hs].rearrange("p (dc c) -> p dc c", c=CB),
                wkt.rearrange("(dc p) c -> p dc c", p=128)[:, hd, :])
            for dc in range(half * 4, half * 4 + 4):
                nc.scalar.dma_start(xts[dc][:, 0:512],
                                    xt[dc * 128:(dc + 1) * 128, 0:512])
        nc.scalar.dma_start(wv_b[:].rearrange("p (dc c) -> p dc c", c=CB),
                            wvt.rearrange("(dc p) c -> p dc c", p=128))
        nc.scalar.dma_start(wq_b[:].rearrange("p (dc c) -> p dc c", c=CB),
                            wqt.rearrange("(dc p) c -> p dc c", p=128))
        hst = per.tile([128, KCH], FP, name="hst")
        nc.scalar.dma_start(hst[:], hv)
        negc = per.tile([128, 1], FP, name="negc")
        nc.vector.memset(negc[:], -C_OFF)
        ln14 = per.tile([128, 1], FP, name="ln14")
        nc.vector.memset(ln14[:], float(np.log(0.25)))
        # 0.25*exp(-h) = exp(-h + ln(0.25))
        nc.scalar.activation(w_s2[:], hst[:], AF.Exp, scale=-1.0,
                             bias=ln14[:, 0:1])
        nc.sync.dma_start(bq_s[:], bqv.unsqueeze(1))
        nc.scalar.dma_start(bk_s[:], bkv.unsqueeze(1))
        nc.scalar.dma_start(bv_s[:], bvv.unsqueeze(1))

        # ------- phases 1+2 fully woven ------------------------------------
        # Section A (q-blocks 0-2) weaves the projections and uses a 2-deep
        # pss rotation; section B (q-blocks 3-7) reclaims the projection PSUM
        # for a 3-deep pss rotation that decouples the exp engines from the
        # S-matmul chain.
        with tc.tile_pool(name="p8s" + sfx, bufs=5) as p8s, \
             tc.tile_pool(name="paos" + sfx, bufs=2) as paos, \
             tc.tile_pool(name="p2po", bufs=1, space="PSUM") as p2po:
          st = {"kv": 1, "qp": 1}
          pending = deque()
          norm_todo = []

          def emit_pv():
              pending.popleft()()

          def mk_dr(pso_, idx_, p8, pb8v_):
              def go():
                  first = idx_[0] == 0
                  idx_[0] += 2
                  last = idx_[0] == KCH
                  for h in range(2):
                      nc.tensor.matmul(
                          pso_[h][:], vw8v[:, p8, :, h * 65:(h + 1) * 65],
                          pb8v_[:, :, h * 512:(h + 1) * 512],
                          start=first, stop=last, perf_mode=DR)
              return go

          def normalize(pso_, q5_, sec_b=False):
              qc = slice(q5_ * 512, (q5_ + 1) * 512)
              snaps, rcs, bchs = [], [], []
              for h in range(2):
                  snap = paos.tile([65, 512], FP, name=f"sn{h}")
                  if sec_b:
                      nc.scalar.copy(snap[:], pso_[h][:])
                  else:
                      nc.vector.tensor_copy(snap[:], pso_[h][:])
                  snaps.append(snap)
              for h in range(2):
                  rc = paos.tile([1, 512], FP, name=f"rc{h}")
                  nc.vector.reciprocal(rc[:], snaps[h][64:65, :])
                  rcs.append(rc)
              for h in range(2):
                  bch = paos.tile([64, 512], FP, name=f"bc{h}")
                  nc.gpsimd.partition_broadcast(bch[:], rcs[h][:])
                  bchs.append(bch)
              for h in range(2):
                  aoh = paos.tile([64, 512], E4, name=f"ao{h}")
                  nc.vector.tensor_tensor(aoh[:], snaps[h][0:64, :], bchs[h][:],
                                          ALU.mult)
                  nc.sync.dma_start(
                      cc_in[q5_ * 128 + h * 64: q5_ * 128 + (h + 1) * 64, :],
                      aoh[:])
                  if dbg is not None:
                      nc.sync.dma_start(
                          dbg["d_ao"][h * 64:(h + 1) * 64, qc], aoh[:])

          def run_q5(q5, pss_pool, weave, sec_b=False):
              qcol = slice(q5 * 512, (q5 + 1) * 512)
              pso = [p2po.tile([65, 512], FP, name=f"pso{h}") for h in range(2)]
              pb8 = None
              pv_idx = [0]
              for kc in range(KCH):
                  krng = slice(kc * 128, (kc + 1) * 128)
                  cls = kc_cls(kc, sec_b)
                  pss = pss_pool.tile([128, 1024], FP, name="pss")
                  for h in range(2):
                      hr = slice(h * 64, (h + 1) * 64)
                      nc.tensor.matmul(pss[:, h * 512:(h + 1) * 512],
                                       kb_t[hr, krng], qb_t[hr, qcol],
                                       start=True, stop=True)
                  p8, j = kc_pair_slot(kc)
                  if j == 0:
                      pb8 = p8s.tile([128, 2048], E4, name="pb8")
                  pb8v = pb8[:].rearrange("p (j n) -> p j n", j=2)
                  if cls == "A":
                      nc.scalar.activation(pb8v[:, j, :], pss[:], AF.Exp,
                                           bias=negc[:, 0:1])
                  else:
                      # e4m3 Schraudolph: bits = max(s * 8/ln2, 0); the
                      # exponent-bias term is folded into C_OFF so two ops
                      # suffice and negatives clamp to +0
                      nc.vector.tensor_scalar(
                          pb8v[:, j, :].bitcast(mybir.dt.uint8), pss[:],
                          float(A8), 0.0, ALU.mult, ALU.max)
                  if j == 1:
                      pending.append(mk_dr(pso, pv_idx, p8, pb8v))
                  while len(pending) > PV_LAG:
                      emit_pv()
                  # previous q-block's normalize: its last P@V flushed by
                  # kc==5, before this block's first P@V is emitted
                  if kc == 5 and norm_todo:
                      normalize(*norm_todo.pop(), sec_b=sec_b)
                  if weave:
                      if q5 == 0:
                          nb = st["kv"]
                          if nb < NB:
                              ph = kc % 4
                              if ph == 0:
                                  if kc // 4 + 2 < NB:
                                      block_load(kc // 4 + 2)
                                  kv_k(nb, 0)
                              elif ph == 1:
                                  kv_k(nb, 1)
                                  kv_v(nb, 0)
                              elif ph == 2:
                                  kv_v(nb, 1)
                                  kv_vt(nb, 0)
                              else:
                                  kv_vt(nb, 1)
                                  st["kv"] += 1
                          elif kc in (28, 29) and st["qp"] < NB:
                              qp_q(st["qp"], kc - 28)
                              if kc == 29:
                                  st["qp"] += 1
                      if q5 in (1, 2):
                          if kc % 5 == 3 and kc < 28 and st["qp"] < NB:
                              qp_q(st["qp"], 0)
                          if kc % 5 == 4 and kc < 29 and st["qp"] < NB:
                              qp_q(st["qp"], 1)
                              st["qp"] += 1
              if q5 < N // 512 - 1:
                  norm_todo.append((pso, q5))
              else:
                  while pending:
                      emit_pv()
                  normalize(pso, q5, sec_b=sec_b)

          with tc.tile_pool(name="p1s" + sfx, bufs=2) as p1s, \
             tc.tile_pool(name="p1p", bufs=1, space="PSUM") as p1p, \
             tc.tile_pool(name="pfp" + sfx, bufs=2) as pfp, \
             tc.tile_pool(name="p2ps", bufs=2, space="PSUM") as p2ps:

            def block_load(nb):
                ncol = slice(nb * 512, (nb + 1) * 512)
                for dc in range(8):
                    nc.sync.dma_start(
                        xts[dc][:, ncol],
                        xt[dc * 128:(dc + 1) * 128, nb * 512:(nb + 1) * 512])

            wstate = {}

            def proj_half(nb, wsrc, half, nm):
                ncol = slice(nb * 512, (nb + 1) * 512)
                if half == 0:
                    # k and q chains share one PSUM slot; v uses the pvt slot
                    wstate[nm] = p1p.tile([128, 512], FP,
                                          name="pvt" if nm == "v" else "proj")
                ps = wstate[nm]
                for dc in range(half * 4, half * 4 + 4):
                    nc.tensor.matmul(ps[:], wsrc[:, dc * CB:(dc + 1) * CB],
                                     xts[dc][:, ncol],
                                     start=(dc == 0), stop=(dc == 7))
                return ps

            def kv_k(nb, half):
                ps = proj_half(nb, wk_b, half, "k")
                if half == 1:
                    nc.scalar.add(kb_t[:, nb * 512:(nb + 1) * 512], ps[:],
                                  bk_s[:, 0:1])

            def kv_v(nb, half):
                ps = proj_half(nb, wv_b, half, "v")
                if half == 1:
                    vtb = p1s.tile([128, 512], BF, name="vtb")
                    nc.scalar.add(vtb[:], ps[:], bv_s[:, 0:1])
                    wstate["vtb"] = vtb

            def kv_vt(nb, half):
                vtb = wstate["vtb"]
                for ns in range(half * 2, half * 2 + 2):
                    kc = nb * 4 + ns
                    pvt = p1p.tile([128, 128], BF, name="pvt")
                    nc.tensor.transpose(pvt[:], vtb[:, ns * 128:(ns + 1) * 128], ident[:])
                    wcol = w_s2[:, kc:kc + 1]
                    p8, j = kc_pair_slot(kc)
                    base = (p8 * 2 + j) * 144
                    dst = vw8[:, base:base + 130]
                    nc.vector.tensor_scalar_mul(dst[:, 0:64], pvt[:, 0:64], wcol)
                    nc.vector.tensor_copy(dst[:, 64:65], wcol)
                    nc.vector.tensor_scalar_mul(dst[:, 65:129], pvt[:, 64:128], wcol)
                    nc.vector.tensor_copy(dst[:, 129:130], wcol)

            def qp_q(nb, half):
                ps = proj_half(nb, wq_b, half, "q")
                if half == 1:
                    nc.vector.tensor_scalar_add(qb_t[:, nb * 512:(nb + 1) * 512],
                                                ps[:], bq_s[:, 0:1])

            block_load(1)
            for hf in range(2):
                kv_k(0, hf)
            for hf in range(2):
                kv_v(0, hf)
            for hf in range(2):
                kv_vt(0, hf)
            for hf in range(2):
                qp_q(0, hf)

            # prefetch phase-3 constants (issued on the mostly-idle gpsimd
            # queue); transfers overlap the attention phase
            nc.gpsimd.dma_start(wo_b[:], wot)
            bost = pfp.tile([128, D], FP, name="bost")
            nc.gpsimd.dma_start(bost[:], bov.unsqueeze(0).broadcast_to([128, D]))
            for j in range(4):
                xrt = pfp.tile([128, D], FP, name="xrt")
                nc.gpsimd.dma_start(xrt[:], xr[j * 128:(j + 1) * 128, :])
                nc.vector.tensor_add(xb_s[j][:], xrt[:], bost[:])

            for q5 in range(3):
                run_q5(q5, p2ps, True)

          with tc.tile_pool(name="p2psб".replace("б", "3"), bufs=3,
                            space="PSUM") as p2ps3:
            for q5 in range(3, N // 512):
                run_q5(q5, p2ps3, False, sec_b=True)

        # ---------------- phase 3: A2A + out projection ----------------
        with tc.tile_pool(name="p3s" + sfx, bufs=1) as p3s, \
             tc.tile_pool(name="p3f" + sfx, bufs=3) as p3f, \
             tc.tile_pool(name="p3p", bufs=1, space="PSUM") as p3p:
            if use_collective:
                nc.gpsimd.collective_compute(
                    "AllToAll", mybir.AluOpType.bypass,
                    replica_groups=[list(range(NCORE))],
                    ins=[cc_in[:].opt()], outs=[cc_out[:].opt()])
            else:  # single-core timing-sim stand-in
                nc.sync.dma_start(cc_out[:], cc_in[:])
            psf = [p3p.tile([128, 512], FP, name=f"psf{t}") for t in range(8)]
            # aoc pairs interleaved for DoubleRow: [128, 2, 512] per cc pair
            aocs = []
            for cp in range(4):
                aoc = p3s.tile([128, 2, NR], E4, name=f"aoc{cp}")
                for jj in range(2):
                    cc = cp * 2 + jj
                    (nc.sync if cc % 2 == 0 else nc.scalar).dma_start(
                        aoc[:, jj, :], cc_out[cc * 128:(cc + 1) * 128, :])
                    if dbg is not None:
                        nc.sync.dma_start(
                            dbg["d_aoc"][cc * 128:(cc + 1) * 128, :], aoc[:, jj, :])
                aocs.append(aoc)
            wo8v = wo_b[:].rearrange("p (cp j o) -> p cp j o", cp=4, j=2)
            for ns in range(4):
                for ob in range(2):
                    t = ns * 2 + ob
                    for cp in range(4):
                        nc.tensor.matmul(
                            psf[t][:],
                            aocs[cp][:, :, ns * 128:(ns + 1) * 128],
                            wo8v[:, cp, :, ob * 512:(ob + 1) * 512],
                            start=(cp == 0), stop=(cp == 3), perf_mode=DR)
                    fo = p3f.tile([128, 512], FP, name="fo")
                    nc.vector.tensor_add(fo[:], psf[t][:],
                                         xb_s[ns][:, ob * 512:(ob + 1) * 512])
                    nc.sync.dma_start(
                        out[ns * 128:(ns + 1) * 128, ob * 512:(ob + 1) * 512], fo[:])

    if dbg is not None:
        for nm, t in (("d_q", qb_t), ("d_k", kb_t)):
            nc.sync.dma_start(dbg[nm], t[:])
        nc.sync.dma_start(dbg["d_vw8"], vw8[:].bitcast(mybir.dt.uint8))
        nc.sync.dma_start(dbg["d_cc"], cc_out[:])

    dram.release()
    per.release()
    cst.release()


def build_nc(reps=1, debug=False, use_collective=True):
    nc = bacc.Bacc("TRN2", target_bir_lowering=False, debug=False, num_devices=NCORE)
    xt = nc.dram_tensor("xt", [D, N], BF, kind="ExternalInput").ap()
    xr = nc.dram_tensor("xr", [NR, D], FP, kind="ExternalInput").ap()
    wqt = nc.dram_tensor("wqt", [D, CB], BF, kind="ExternalInput").ap()
    wkt = nc.dram_tensor("wkt", [D, CB], BF, kind="ExternalInput").ap()
    wvt = nc.dram_tensor("wvt", [D, CB], BF, kind="ExternalInput").ap()
    wot = nc.dram_tensor("wot", [128, 8 * D], E4, kind="ExternalInput").ap()
    bqv = nc.dram_tensor("bqv", [CB], FP, kind="ExternalInput").ap()
    bkv = nc.dram_tensor("bkv", [CB], FP, kind="ExternalInput").ap()
    bvv = nc.dram_tensor("bvv", [CB], FP, kind="ExternalInput").ap()
    bov = nc.dram_tensor("bov", [D], FP, kind="ExternalInput").ap()
    # h pre-rearranged on host to [128, KCH]: hv[p, c] = h[c*128 + p]
    hv = nc.dram_tensor("hv", [128, KCH], FP, kind="ExternalInput").ap()
    out = nc.dram_tensor("out", [NR, D], FP, kind="ExternalOutput").ap()
    dbg = None
    if debug:
        dbg = {
            "d_q": nc.dram_tensor("d_q", [128, N], BF, kind="ExternalOutput").ap(),
            "d_k": nc.dram_tensor("d_k", [128, N], BF, kind="ExternalOutput").ap(),
            "d_vw8": nc.dram_tensor("d_vw8", [128, 16 * 2 * 144], mybir.dt.uint8,
                                    kind="ExternalOutput").ap(),
            "d_ao": nc.dram_tensor("d_ao", [128, N], E4, kind="ExternalOutput").ap(),
            "d_cc": nc.dram_tensor("d_cc", [NCORE * 128, NR], E4,
                                   kind="ExternalOutput").ap(),
            "d_aoc": nc.dram_tensor("d_aoc", [NCORE * 128, NR], E4,
                                    kind="ExternalOutput").ap(),
        }
    with tile.TileContext(nc) as tc:
        _body(nc, tc, reps, xt, xr, wqt, wkt, wvt, wot,
              bqv, bkv, bvv, bov, hv, out, dbg=dbg, use_collective=use_collective)
    nc.compile()
    return nc


_NC_CACHE = {}


def get_nc(reps=1):
    if reps not in _NC_CACHE:
        _NC_CACHE[reps] = build_nc(reps)
    return _NC_CACHE[reps]


def make_in_maps(inputs):
    x = np.ascontiguousarray(np.asarray(inputs["x"], dtype=np.float32))
    h = np.ascontiguousarray(np.asarray(inputs["h"], dtype=np.float32))
    Wq = np.asarray(inputs["Wq"], dtype=np.float32)
    bq = np.asarray(inputs["bq"], dtype=np.float32)
    Wk = np.asarray(inputs["Wk"], dtype=np.float32)
    bk = np.asarray(inputs["bk"], dtype=np.float32)
    Wv = np.asarray(inputs["Wv"], dtype=np.float32)
    bv = np.asarray(inputs["bv"], dtype=np.float32)
    Wo = np.asarray(inputs["Wo"], dtype=np.float32)
    bo = np.ascontiguousarray(np.asarray(inputs["bo"], dtype=np.float32))
    import ml_dtypes
    bf16 = ml_dtypes.bfloat16
    import ml_dtypes as _md
    e4m3 = _md.float8_e4m3fn
    xt = np.ascontiguousarray(x.T.astype(bf16))
    # Wo^T rows DR-interleaved by cc pairs: [128, (pair, j, o)]
    wot_r = Wo.T.reshape(4, 2, 128, D)          # [pair, j, 128, o]
    wot = np.ascontiguousarray(
        np.clip(wot_r.transpose(2, 0, 1, 3), -240, 240).astype(e4m3)
        .reshape(128, 8 * D))
    scale = np.float32(0.125)  # 1/sqrt(dh), folded into q
    in_maps = []
    for i in range(NCORE):
        cs = slice(i * CB, (i + 1) * CB)
        in_maps.append({
            "xt": xt,
            "xr": np.ascontiguousarray(x[i * NR:(i + 1) * NR, :]),
            "wqt": np.ascontiguousarray((Wq[cs, :] * scale).T.astype(bf16)),
            "wkt": np.ascontiguousarray(Wk[cs, :].T.astype(bf16)),
            "wvt": np.ascontiguousarray(Wv[cs, :].T.astype(bf16)),
            "wot": wot,
            "bqv": np.ascontiguousarray(bq[cs] * scale),
            "bkv": np.ascontiguousarray(bk[cs]),
            "bvv": np.ascontiguousarray(bv[cs]),
            "bov": bo,
            "hv": np.ascontiguousarray(h.reshape(KCH, 128).T),
        })
    return in_maps


def kernel(**inputs):
    nc = get_nc(1)
    in_maps = make_in_maps(inputs)
    res = run_bass_kernel_spmd(nc, in_maps, core_ids=list(range(NCORE)))
    return np.concatenate([res.results[i]["out"] for i in range(NCORE)], axis=0)
